# revision 54
# baseline (speedup 1.0000x reference)
"""Trainium2 Bass kernel for EruSelfAttentionModel.

Math (reference, simplified):
  e  = emb_table[x]                                  # [B,S,E] gather
  h  = LayerNorm(e) * gamma + beta                   # over E
  q  = einsum('hae,bse->bhsa', Wq, h); k likewise    # A=64 per head
  v  = einsum('hve,bse->bhsv', Wv, h)                # v-dim = E
  scores = q @ k^T / sqrt(E)
  sn = (scores - min) / (max - min)  (rowwise)
  softmax_sel = 1 - max(sn) == 0 exactly  =>  weights = sigmoid(10*sn - 5)
  out = weights @ v                                  # [B,H,S,E]

Key identities used:
  - sn is invariant to positive rescaling of scores => the 1/sqrt(E) scale
    can be dropped entirely.
  - weights = sigmoid(alpha * scores + beta_row) with per-row
    alpha = 10/(mx-mn), beta_row = -10*mn/(mx-mn) - 5  => single fused
    ScalarE activation pass (per-partition scale/bias APs).

Layouts (per core = one batch):
  - token t <-> (partition p, chunk c) with t = 8p + c (set by the gather's
    idx permutation); all of kT/vh/wt use this same permutation so the
    contraction over t is consistent.
  - e-dim (for h^T):  e = 128*c2 + p  (the SBUF dma xbar transpose is
    partition-minor: out[p, c2, s] = in[s, 128*c2 + p]), identical to the
    PE-transpose layout, so weights keep the "(ec p) j" host layout.
  - fp8 out-matmul (DoubleRow): W transposed in bf16 by the xbar
    (wt[p, cc, s] = W[s, 128*cc + p], i.e. token 8p+cc), then cast to fp8
    by a DVE copy. DoubleRow pair dim = chunk pairs cc in {2j, 2j+1}:
    lhsT = wt8[:, 2j:2j+2, :] pairs tokens (8p+2j, 8p+2j+1), matching
    rhs = vh[:, 2j:2j+2, :] in the same partition.

Sharding: data-parallel over batch; core b computes batch b fully.
"""

import os
import sys

sys.path.insert(0, "/opt/trn_rl_repo")

import numpy as np
import ml_dtypes

import concourse.bass as bass
import concourse.bacc as bacc
import concourse.tile as tile
from concourse import mybir
from concourse.bass_utils import run_bass_kernel_spmd
import concourse.bass_utils as _bass_utils

# Let walrus overlap LDWEIGHTS with in-flight matmuls (the PE pulls LDW for
# the background weight buffer ahead); without it every matmul serializes
# behind its weight load.
if not getattr(_bass_utils, "_ldw_patched", False):
    _orig_run_command = _bass_utils.run_command

    def _patched_run_command(argv, **kwargs):
        if os.environ.get("KERNEL_LDW_OPT", "0") == "1":
            argv = [
                a.replace("--enable-ldw-opt=false", "--enable-ldw-opt=true")
                if isinstance(a, str) else a
                for a in argv
            ]
        return _orig_run_command(argv, **kwargs)

    _bass_utils.run_command = _patched_run_command
    _bass_utils._ldw_patched = True

BF16 = ml_dtypes.bfloat16

VOCAB, E, A, H = 32000, 512, 64, 8
B, S = 8, 1024
P = 128                 # partitions
NCH = S // P            # 8 token chunks
EC = E // P             # 4 embedding chunks
LN_EPS = 1e-5

F32 = mybir.dt.float32
BF = mybir.dt.bfloat16
FP8 = mybir.dt.float8e4
I16 = mybir.dt.int16

USE_FP8_OUT = os.environ.get("KERNEL_FP8", "1") == "1"

_BUILD_CACHE = {}
LAST_RESULTS = None     # test.py reads exec_time_ns from here


def build_nc(use_beta: bool, use_fp8: bool):
    key = (use_beta, use_fp8)
    if key in _BUILD_CACHE:
        return _BUILD_CACHE[key]

    nc = bacc.Bacc("TRN2", target_bir_lowering=False, num_devices=8)

    idx_d = nc.declare_dram_parameter("idx", [P, S // 16], I16, isOutput=False)
    emb_d = nc.declare_dram_parameter("emb", [VOCAB, E], F32, isOutput=False)
    wqt_d = nc.declare_dram_parameter("wqt", [E, H * A], BF, isOutput=False)
    wkt_d = nc.declare_dram_parameter("wkt", [E, H * A], BF, isOutput=False)
    wvt_d = nc.declare_dram_parameter("wvt", [E, H * E], BF, isOutput=False)
    if use_beta:
        qb_d = nc.declare_dram_parameter("qb", [P, 4], F32, isOutput=False)
        kb_d = nc.declare_dram_parameter("kb", [P, 4], F32, isOutput=False)
        vb_d = nc.declare_dram_parameter("vb", [1, H * E], F32, isOutput=False)
    out_d = nc.declare_dram_parameter("out", [H, S, E], BF, isOutput=True)

    V_DT = FP8 if use_fp8 else BF
    wtb_bufs = 10 if use_fp8 else 12
    keep_pending = 8 if use_fp8 else 5

    with tile.TileContext(nc) as tc:
        with tc.tile_pool(name="consts", bufs=1) as consts:
            idx_sb = consts.tile([P, S // 16], I16)
            nc.sync.dma_start(idx_sb[:], idx_d[:])
            wqt_sb = consts.tile([P, EC, H * A], BF)
            nc.sync.dma_start(
                wqt_sb[:], wqt_d.ap().rearrange("(ec p) j -> p ec j", p=P)
            )
            wkt_sb = consts.tile([P, EC, H * A], BF)
            nc.sync.dma_start(
                wkt_sb[:], wkt_d.ap().rearrange("(ec p) j -> p ec j", p=P)
            )
            wvt_sb = consts.tile([P, EC, H * E], BF)
            # split the 4MB load by hv-column group so the first vhat can
            # start as soon as its columns land
            wvt_r = wvt_d.ap().rearrange("(ec p) j -> p ec j", p=P)
            for vq in range(4):
                nc.sync.dma_start(
                    wvt_sb[:, :, vq * 1024 : (vq + 1) * 1024],
                    wvt_r[:, :, vq * 1024 : (vq + 1) * 1024],
                )
            eps_sb = consts.tile([P, 1], F32)
            nc.vector.memset(eps_sb[:], LN_EPS)

            if use_beta:
                qb_sb = consts.tile([P, 4], F32)
                nc.sync.dma_start(qb_sb[:], qb_d[:])
                kb_sb = consts.tile([P, 4], F32)
                nc.sync.dma_start(kb_sb[:], kb_d[:])
                vb_sb = consts.tile([P, H * E], F32)
                vb_bcast = bass.AP(
                    tensor=vb_d, offset=0, ap=[[0, P], [1, H * E]]
                )
                nc.sync.dma_start(vb_sb[:], vb_bcast)

            # persistent activations
            hT_sb = consts.tile([P, EC, S], BF)       # hT[p, c2, s] = h^T[128*c2+p, tok(s)]
            qT_sb = consts.tile([P, EC, S], BF)       # qT[ha%128, ha//128, s]
            kT_sb = consts.tile([P, EC, S], BF)
            vh_sb = consts.tile([P, NCH, H * E], V_DT)  # vh[p, c, v] = V[8p+c, v] (*0.5 if fp8)
            if use_fp8:
                # centered-W scheme: out = T @ (V/2) + bcast(0.5*colsum(V)),
                # T = tanh((alpha*s+beta)/2) = 2*(sigmoid(alpha*s+beta)-0.5)
                hsum_f = consts.tile([P, EC], F32)
                hsum_b = consts.tile([P, EC], BF)
                csum_sb = consts.tile([1, H * E], BF)   # 0.5 * colsum(V)
                ones1_sb = consts.tile([1, P], BF)
                nc.vector.memset(ones1_sb[:], 1.0)

            # ---------------- phase A: gather + LN + transpose ----------------
            with (
                tc.tile_pool(name="e_pool", bufs=1) as e_pool,
                tc.tile_pool(name="h_pool", bufs=3) as h_pool,
                tc.tile_pool(name="st_pool", bufs=8) as st_pool,
            ):
                # per-chunk LN so chunk 0's h (and the first vhat groups)
                # flow while later chunks still gather
                mv = st_pool.tile([P, NCH, 2], F32, tag="mv")
                for c in range(NCH):
                    e_t = e_pool.tile([P, 1, E], F32, tag=f"e{c}")
                    nc.gpsimd.dma_gather(
                        e_t[:], emb_d.ap(), idx_sb[:, 8 * c : 8 * (c + 1)],
                        P, P, E,
                    )
                    stt = st_pool.tile([P, 6], F32, tag="bn")
                    nc.vector.bn_stats(stt[:], e_t[:, 0, :])
                    nc.vector.bn_aggr(mv[:, c, :], stt[:])
                    var_ap = mv[:, c, 1:2]
                    nc.scalar.activation(
                        out=var_ap, in_=var_ap,
                        func=mybir.ActivationFunctionType.Sqrt,
                        bias=eps_sb[:, 0:1], scale=1.0,
                    )
                    nc.vector.reciprocal(var_ap, var_ap)
                    h_t = h_pool.tile([P, E], BF)
                    nc.vector.tensor_scalar(
                        out=h_t[:], in0=e_t[:, 0, :],
                        scalar1=mv[:, c, 0:1], scalar2=mv[:, c, 1:2],
                        op0=mybir.AluOpType.subtract, op1=mybir.AluOpType.mult,
                    )
                    # hT[p, c2, 128c+s] = h_t[s, 128*c2+p] via xbar transpose
                    nc.sync.dma_start_transpose(
                        hT_sb[:, :, c * P : (c + 1) * P], h_t[:]
                    )

            # ---------------- phases B+C: projections + attention ----------
            with (
                tc.tile_pool(name="sc_psum", bufs=3, space="PSUM") as sc_psum,
                tc.tile_pool(name="out_psum", bufs=2, space="PSUM") as out_psum,
                tc.tile_pool(name="sstat", bufs=10) as sstat,
                tc.tile_pool(name="w_pool", bufs=4) as w_pool,
                tc.tile_pool(name="wraw_pool", bufs=20) as wraw_pool,
                tc.tile_pool(name="wtb_pool", bufs=wtb_bufs) as wtb_pool,
                tc.tile_pool(name="wt_pool", bufs=14) as wt_pool,
                tc.tile_pool(name="ob_pool", bufs=6) as ob_pool,
            ):
                dum_bf = sstat.tile([P, S], BF, tag="dum", bufs=1)

                def vhat_group(c):
                    # V projection for t-group c, interleaved into phase C
                    # to keep TensorE dense.
                    for vp in range(4):
                        pv = sc_psum.tile([P, S], F32, tag="sc")
                        for c2 in range(EC):
                            lhsT = hT_sb[:, c2, c * P : (c + 1) * P]
                            for nn in range(2):
                                lo = vp * 1024 + nn * 512
                                nc.tensor.matmul(
                                    pv[:, nn * 512 : (nn + 1) * 512],
                                    lhsT,
                                    wvt_sb[:, c2, lo : lo + 512],
                                    start=(c2 == 0), stop=(c2 == EC - 1),
                                )
                        if use_fp8:
                            nc.scalar.mul(
                                vh_sb[:, c, vp * 1024 : (vp + 1) * 1024], pv[:], 0.5
                            )
                        else:
                            nc.scalar.copy(
                                vh_sb[:, c, vp * 1024 : (vp + 1) * 1024], pv[:]
                            )

                def out_stage(args):
                    hp_, i_, wts = args
                    for sub in range(2):
                        h_idx = 2 * hp_ + sub
                        po = out_psum.tile([P, E], F32, tag="po")
                        if use_fp8:
                            # wts is one packed uint16 tile; fp8 view with
                            # byte b = sub: [p][cc][sub][s]
                            w4 = wts[:].bitcast(FP8).rearrange(
                                "p c (s two) -> p c two s", two=2
                            )
                            for j in range(4):
                                lhsT = w4[:, 2 * j : 2 * j + 2, sub, :]
                                rhs = vh_sb[
                                    :, 2 * j : 2 * j + 2,
                                    h_idx * E : (h_idx + 1) * E,
                                ]
                                nc.tensor.matmul(
                                    po[:], lhsT, rhs,
                                    start=(j == 0), stop=False,
                                    perf_mode=mybir.MatmulPerfMode.DoubleRow,
                                )
                            # += ones (x) 0.5*colsum(V): the centering term
                            nc.tensor.matmul(
                                po[:], ones1_sb[0:1, :],
                                csum_sb[0:1, h_idx * E : (h_idx + 1) * E],
                                start=False, stop=True,
                                skip_group_check=True,
                            )
                        else:
                            wt_t = wts[sub]
                            for cc in range(NCH):
                                nc.tensor.matmul(
                                    po[:],
                                    wt_t[:, cc, :],
                                    vh_sb[:, cc, h_idx * E : (h_idx + 1) * E],
                                    start=(cc == 0), stop=(cc == NCH - 1),
                                )
                        ob = ob_pool.tile([P, E], BF, tag="ob")
                        # alternate drain engines so two po tiles recycle
                        # in parallel
                        if sub == 0:
                            nc.scalar.copy(ob[:], po[:])
                        else:
                            nc.vector.tensor_copy(ob[:], po[:])
                        # SWDGE store: keeps the sync-engine queue free for
                        # the latency-critical wt transposes
                        nc.gpsimd.dma_start(
                            out_d[h_idx, i_ * P : (i_ + 1) * P, :], ob[:]
                        )

                # early V-hat groups: each needs only its own hT chunk, so
                # these overlap the tail of phase A and warm the PE
                for c in range(3):
                    vhat_group(c)

                # projections q/k
                for name, w_sb, t_sb in (
                    ("q", wqt_sb, qT_sb),
                    ("k", wkt_sb, kT_sb),
                ):
                    for sl in range(4):  # 128-wide (h,a) slices = head pairs
                        pq = sc_psum.tile([P, S], F32, tag="sc")
                        for c2 in range(EC):
                            lhsT = w_sb[:, c2, sl * P : (sl + 1) * P]
                            for nn in range(2):
                                nc.tensor.matmul(
                                    pq[:, nn * 512 : (nn + 1) * 512],
                                    lhsT,
                                    hT_sb[:, c2, nn * 512 : (nn + 1) * 512],
                                    start=(c2 == 0), stop=(c2 == EC - 1),
                                )
                        if use_beta:
                            bb = qb_sb if name == "q" else kb_sb
                            nc.vector.tensor_scalar(
                                out=t_sb[:, sl, :], in0=pq[:],
                                scalar1=bb[:, sl : sl + 1], scalar2=None,
                                op0=mybir.AluOpType.add, op1=mybir.AluOpType.bypass,
                            )
                        else:
                            nc.scalar.copy(t_sb[:, sl, :], pq[:])

                if use_fp8:
                    # 0.5*colsum(V)[v] = 0.5 * sum_e (sum_t h[t,e]) * WvT[e,v]
                    nc.vector.tensor_reduce(
                        hsum_f[:], hT_sb[:], axis=mybir.AxisListType.X,
                        op=mybir.AluOpType.add,
                    )
                    nc.vector.tensor_copy(hsum_b[:], hsum_f[:])
                    for j in range(8):
                        cs_ps = out_psum.tile([1, 512], F32, tag="po")
                        for c2 in range(EC):
                            nc.tensor.matmul(
                                cs_ps[:],
                                hsum_b[:, c2 : c2 + 1],
                                wvt_sb[:, c2, j * 512 : (j + 1) * 512],
                                start=(c2 == 0), stop=(c2 == EC - 1),
                            )
                        nc.scalar.mul(
                            csum_sb[0:1, j * 512 : (j + 1) * 512], cs_ps[:], 0.5
                        )

                pending = []
                unit_no = 0
                # pairs: groups of 2 query-chunk units sharing one stats batch
                for q in range(16):
                    hp = q // 4
                    i0 = 2 * (q % 4)
                    st = sstat.tile([P, 2, 8], F32, tag="st")
                    # st[:, u, :]: [mxA, mxB, mnA, mnB, betA, betB, alpA, alpB]
                    wraws = {}
                    for u in range(2):
                        i = i0 + u
                        if unit_no + 3 < NCH:
                            vhat_group(unit_no + 3)
                        unit_no += 1
                        ps = []
                        for sub in range(2):
                            p0 = sub * 64
                            psc = sc_psum.tile([P, S], F32, tag="sc")
                            for nn in range(2):
                                nc.tensor.matmul(
                                    psc[:, nn * 512 : (nn + 1) * 512],
                                    qT_sb[p0 : p0 + 64, hp, i * P : (i + 1) * P],
                                    kT_sb[p0 : p0 + 64, hp, nn * 512 : (nn + 1) * 512],
                                    start=True, stop=True,
                                )
                            ps.append(psc)
                        for sub in range(2):
                            wraw = wraw_pool.tile([P, S], BF, tag="wr")
                            # fused PSUM->SBUF copy + row-max accum
                            nc.vector.tensor_scalar(
                                out=wraw[:], in0=ps[sub][:],
                                scalar1=-3.0e38, scalar2=None,
                                op0=mybir.AluOpType.max, op1=mybir.AluOpType.max,
                                accum_out=st[:, u, sub : sub + 1],
                            )
                            wraws[(u, sub)] = wraw
                        for sub in range(2):
                            # two-stage row-min: bf16 TT-min of halves (2x),
                            # then min-accum over the 512-wide result
                            wr = wraws[(u, sub)]
                            nc.vector.tensor_tensor(
                                out=dum_bf[:, 0:512], in0=wr[:, 0:512],
                                in1=wr[:, 512:1024], op=mybir.AluOpType.min,
                            )
                            nc.vector.tensor_scalar(
                                out=dum_bf[:, 512:1024], in0=dum_bf[:, 0:512],
                                scalar1=3.0e38, scalar2=None,
                                op0=mybir.AluOpType.min, op1=mybir.AluOpType.min,
                                accum_out=st[:, u, 2 + sub : 3 + sub],
                            )
                        keep = keep_pending if q < 14 else 2
                        while len(pending) > keep:
                            out_stage(pending.pop(0))
                    # batched alpha/beta for the 4 subs of this pair batch:
                    # sigmoid: alpha = 10/(mx-mn); beta = -mn*alpha - 5
                    # tanh/2 (fp8): alpha = 5/(mx-mn); beta = -mn*alpha - 2.5
                    amul = 5.0 if use_fp8 else 10.0
                    boff = -2.5 if use_fp8 else -5.0
                    mx = st[:, :, 0:2]
                    mn = st[:, :, 2:4]
                    bet = st[:, :, 4:6]
                    alp = st[:, :, 6:8]
                    nc.vector.tensor_tensor(
                        out=alp, in0=mx, in1=mn, op=mybir.AluOpType.subtract
                    )
                    nc.vector.reciprocal(alp, alp)
                    nc.vector.tensor_scalar_mul(alp, alp, amul)
                    nc.vector.tensor_tensor(
                        out=bet, in0=mn, in1=alp, op=mybir.AluOpType.mult
                    )
                    nc.vector.tensor_scalar(
                        out=bet, in0=bet, scalar1=-1.0, scalar2=boff,
                        op0=mybir.AluOpType.mult, op1=mybir.AluOpType.add,
                    )
                    act_fn = (
                        mybir.ActivationFunctionType.Tanh if use_fp8
                        else mybir.ActivationFunctionType.Sigmoid
                    )
                    for u in range(2):
                        i = i0 + u
                        if use_fp8:
                            # both subs' tanh outputs packed as the two bytes
                            # of one uint16 tile; one transpose, no casts
                            pk = w_pool.tile([P, S], I16, tag="pk")
                            pk8 = pk[:].bitcast(FP8).rearrange(
                                "p (s two) -> p two s", two=2
                            )
                            for sub in range(2):
                                nc.scalar.activation(
                                    out=pk8[:, sub, :], in_=wraws[(u, sub)][:],
                                    func=act_fn,
                                    bias=st[:, u, 4 + sub : 5 + sub],
                                    scale=st[:, u, 6 + sub : 7 + sub],
                                )
                            wtp = wt_pool.tile([P, NCH, P], I16, tag="wt")
                            nc.sync.dma_start_transpose(wtp[:], pk[:])
                            pending.append((hp, i, wtp))
                        else:
                            wts = []
                            for sub in range(2):
                                w_t = w_pool.tile([P, S], BF, tag="w")
                                nc.scalar.activation(
                                    out=w_t[:], in_=wraws[(u, sub)][:],
                                    func=act_fn,
                                    bias=st[:, u, 4 + sub : 5 + sub],
                                    scale=st[:, u, 6 + sub : 7 + sub],
                                )
                                wtb = wtb_pool.tile([P, NCH, P], BF, tag="wtb")
                                nc.sync.dma_start_transpose(wtb[:], w_t[:])
                                wts.append(wtb)
                            pending.append((hp, i, wts))
                for pp_ in pending:
                    out_stage(pp_)

    nc.compile()
    _BUILD_CACHE[key] = nc
    return nc


def _prep_inputs(x, emb_table, gamma, beta, Wq, Wk, Wv, use_beta):
    x = np.asarray(x)
    gamma = np.asarray(gamma, dtype=np.float32)
    beta = np.asarray(beta, dtype=np.float32)
    Wq = np.asarray(Wq, dtype=np.float32)
    Wk = np.asarray(Wk, dtype=np.float32)
    Wv = np.asarray(Wv, dtype=np.float32)
    emb = np.ascontiguousarray(np.asarray(emb_table, dtype=np.float32))

    # W'[h,a,e] = W[h,a,e] * gamma[e]; layouts [e, h*ad+a] with the
    # device e-permutation e_dev[p, c2] = e_logical[4p + c2] handled by
    # the rearrange on the DMA load (row-major (p, ec) blocks), so host
    # just provides [e, j] with e in logical order re-grouped as 4p+c2:
    # row index r = 4p + c2 must equal logical e  ->  identity.
    wqt = np.ascontiguousarray(
        (Wq * gamma[None, None, :]).reshape(H * A, E).T.astype(BF16)
    )
    wkt = np.ascontiguousarray(
        (Wk * gamma[None, None, :]).reshape(H * A, E).T.astype(BF16)
    )
    wvt = np.ascontiguousarray(
        (Wv * gamma[None, None, :]).reshape(H * E, E).T.astype(BF16)
    )

    consts = dict(emb=emb, wqt=wqt, wkt=wkt, wvt=wvt)
    if use_beta:
        qb = (Wq.reshape(H * A, E) @ beta).astype(np.float32)   # [512]
        kb = (Wk.reshape(H * A, E) @ beta).astype(np.float32)
        vb = (Wv.reshape(H * E, E) @ beta).astype(np.float32)   # [4096]
        consts["qb"] = np.ascontiguousarray(qb.reshape(4, P).T)
        consts["kb"] = np.ascontiguousarray(kb.reshape(4, P).T)
        consts["vb"] = vb.reshape(1, H * E)

    in_maps = []
    for b in range(B):
        xi = x[b].astype(np.int64)
        idx16 = np.ascontiguousarray(
            xi.reshape(S // 16, 16).T.astype(np.int16)
        )  # [16, 64]; token j of chunk c sits at [j%16, 8c + j//16]
        idx_full = np.ascontiguousarray(np.tile(idx16, (8, 1)))  # [128, 64]
        in_maps.append(dict(idx=idx_full, **consts))
    return in_maps


def kernel(x, emb_table, gamma, beta, Wq, Wk, Wv):
    global LAST_RESULTS
    beta_arr = np.asarray(beta, dtype=np.float32)
    use_beta = bool(np.any(beta_arr != 0.0))

    nc = build_nc(use_beta, USE_FP8_OUT)
    in_maps = _prep_inputs(x, emb_table, gamma, beta, Wq, Wk, Wv, use_beta)

    trace = os.environ.get("KERNEL_TRACE", "0") == "1"
    res = run_bass_kernel_spmd(
        nc, in_maps, core_ids=list(range(B)), trace=trace
    )
    LAST_RESULTS = res

    out = np.stack([np.asarray(res.results[b]["out"]) for b in range(B)], axis=0)
    return out.astype(np.float32)


if __name__ == "__main__":
    rng = np.random.default_rng(0)
    x = rng.integers(0, VOCAB, size=(B, S), dtype=np.int32)
    emb = rng.standard_normal((VOCAB, E), dtype=np.float32)
    gamma = np.ones(E, np.float32)
    beta = np.zeros(E, np.float32)
    Wq = rng.random((H, A, E), dtype=np.float32)
    Wk = rng.random((H, A, E), dtype=np.float32)
    Wv = rng.random((H, E, E), dtype=np.float32)
    out = kernel(x, emb, gamma, beta, Wq, Wk, Wv)
    print(out.shape, out.dtype)


# revision 70
# speedup vs baseline: 1.1076x; 1.1076x over previous
"""Trainium2 Bass kernel for EruSelfAttentionModel.

Math (reference, simplified):
  e  = emb_table[x]                                  # [B,S,E] gather
  h  = LayerNorm(e) * gamma + beta                   # over E
  q  = einsum('hae,bse->bhsa', Wq, h); k likewise    # A=64 per head
  v  = einsum('hve,bse->bhsv', Wv, h)                # v-dim = E
  scores = q @ k^T / sqrt(E)
  sn = (scores - min) / (max - min)  (rowwise)
  softmax_sel = 1 - max(sn) == 0 exactly  =>  weights = sigmoid(10*sn - 5)
  out = weights @ v                                  # [B,H,S,E]

Key identities used:
  - sn is invariant to positive rescaling of scores => the 1/sqrt(E) scale
    can be dropped entirely.
  - weights = sigmoid(alpha * scores + beta_row) with per-row
    alpha = 10/(mx-mn), beta_row = -10*mn/(mx-mn) - 5  => single fused
    ScalarE activation pass (per-partition scale/bias APs).

Layouts (per core = one batch):
  - token t <-> (partition p, chunk c) with t = 8p + c (set by the gather's
    idx permutation); all of kT/vh/wt use this same permutation so the
    contraction over t is consistent.
  - e-dim (for h^T):  e = 128*c2 + p  (the SBUF dma xbar transpose is
    partition-minor: out[p, c2, s] = in[s, 128*c2 + p]), identical to the
    PE-transpose layout, so weights keep the "(ec p) j" host layout.
  - fp8 out-matmul (DoubleRow): W transposed in bf16 by the xbar
    (wt[p, cc, s] = W[s, 128*cc + p], i.e. token 8p+cc), then cast to fp8
    by a DVE copy. DoubleRow pair dim = chunk pairs cc in {2j, 2j+1}:
    lhsT = wt8[:, 2j:2j+2, :] pairs tokens (8p+2j, 8p+2j+1), matching
    rhs = vh[:, 2j:2j+2, :] in the same partition.

Sharding: data-parallel over batch; core b computes batch b fully.
"""

import os
import sys

sys.path.insert(0, "/opt/trn_rl_repo")

import numpy as np
import ml_dtypes

import concourse.bass as bass
import concourse.bacc as bacc
import concourse.tile as tile
from concourse import mybir
from concourse.bass_utils import run_bass_kernel_spmd
import concourse.bass_utils as _bass_utils

# Let walrus overlap LDWEIGHTS with in-flight matmuls (the PE pulls LDW for
# the background weight buffer ahead); without it every matmul serializes
# behind its weight load.
if not getattr(_bass_utils, "_ldw_patched", False):
    _orig_run_command = _bass_utils.run_command

    def _patched_run_command(argv, **kwargs):
        if os.environ.get("KERNEL_LDW_OPT", "0") == "1":
            argv = [
                a.replace("--enable-ldw-opt=false", "--enable-ldw-opt=true")
                if isinstance(a, str) else a
                for a in argv
            ]
        return _orig_run_command(argv, **kwargs)

    _bass_utils.run_command = _patched_run_command
    _bass_utils._ldw_patched = True

BF16 = ml_dtypes.bfloat16

VOCAB, E, A, H = 32000, 512, 64, 8
B, S = 8, 1024
P = 128                 # partitions
NCH = S // P            # 8 token chunks
EC = E // P             # 4 embedding chunks
LN_EPS = 1e-5

F32 = mybir.dt.float32
BF = mybir.dt.bfloat16
FP8 = mybir.dt.float8e4
I16 = mybir.dt.int16

USE_FP8_OUT = os.environ.get("KERNEL_FP8", "1") == "1"

_BUILD_CACHE = {}
LAST_RESULTS = None     # test.py reads exec_time_ns from here


def build_nc(use_beta: bool, use_fp8: bool):
    key = (use_beta, use_fp8)
    if key in _BUILD_CACHE:
        return _BUILD_CACHE[key]

    nc = bacc.Bacc("TRN2", target_bir_lowering=False, num_devices=8)

    idx_d = nc.declare_dram_parameter("idx", [P, S // 16], I16, isOutput=False)
    emb_d = nc.declare_dram_parameter("emb", [VOCAB, E], F32, isOutput=False)
    wqt_d = nc.declare_dram_parameter("wqt", [E, H * A], BF, isOutput=False)
    wkt_d = nc.declare_dram_parameter("wkt", [E, H * A], BF, isOutput=False)
    wvt_d = nc.declare_dram_parameter("wvt", [E, H * E], BF, isOutput=False)
    if use_beta:
        qb_d = nc.declare_dram_parameter("qb", [P, 4], F32, isOutput=False)
        kb_d = nc.declare_dram_parameter("kb", [P, 4], F32, isOutput=False)
        vb_d = nc.declare_dram_parameter("vb", [1, H * E], F32, isOutput=False)
    out_d = nc.declare_dram_parameter("out", [H, S, E], BF, isOutput=True)

    V_DT = FP8 if use_fp8 else BF
    wtb_bufs = 10 if use_fp8 else 12
    keep_pending = 8 if use_fp8 else 5

    with tile.TileContext(nc) as tc:
        with tc.tile_pool(name="consts", bufs=1) as consts:
            idx_sb = consts.tile([P, S // 16], I16)
            nc.sync.dma_start(idx_sb[:], idx_d[:])
            wqt_sb = consts.tile([P, EC, H * A], BF)
            nc.sync.dma_start(
                wqt_sb[:], wqt_d.ap().rearrange("(ec p) j -> p ec j", p=P)
            )
            wkt_sb = consts.tile([P, EC, H * A], BF)
            nc.sync.dma_start(
                wkt_sb[:], wkt_d.ap().rearrange("(ec p) j -> p ec j", p=P)
            )
            wvt_sb = consts.tile([P, EC, H * E], BF)
            # split the 4MB load by hv-column group so the first vhat can
            # start as soon as its columns land
            wvt_r = wvt_d.ap().rearrange("(ec p) j -> p ec j", p=P)
            for vq in range(4):
                nc.sync.dma_start(
                    wvt_sb[:, :, vq * 1024 : (vq + 1) * 1024],
                    wvt_r[:, :, vq * 1024 : (vq + 1) * 1024],
                )
            eps_sb = consts.tile([P, 1], F32)
            nc.vector.memset(eps_sb[:], LN_EPS)
            # dummy activation: pulls the ~2.7us Sqrt table-set load off the
            # chunk-0 LayerNorm chain that gates the kernel's first matmul
            warm_sb = consts.tile([P, 1], F32)
            nc.scalar.activation(
                out=warm_sb[:], in_=eps_sb[:],
                func=mybir.ActivationFunctionType.Sqrt,
                bias=eps_sb[:, 0:1], scale=1.0,
            )

            if use_beta:
                qb_sb = consts.tile([P, 4], F32)
                nc.sync.dma_start(qb_sb[:], qb_d[:])
                kb_sb = consts.tile([P, 4], F32)
                nc.sync.dma_start(kb_sb[:], kb_d[:])
                vb_sb = consts.tile([P, H * E], F32)
                vb_bcast = bass.AP(
                    tensor=vb_d, offset=0, ap=[[0, P], [1, H * E]]
                )
                nc.sync.dma_start(vb_sb[:], vb_bcast)

            # persistent activations
            hT_sb = consts.tile([P, EC, S], BF)       # hT[p, c2, s] = h^T[128*c2+p, tok(s)]
            qT_sb = consts.tile([P, EC, S], BF)       # qT[ha%128, ha//128, s]
            kT_sb = consts.tile([P, EC, S], BF)
            vh_sb = consts.tile([P, NCH, H * E], V_DT)  # vh[p, c, v] = V[8p+c, v] (*0.5 if fp8)
            if use_fp8:
                # centered-W scheme: out = T @ (V/2) + bcast(0.5*colsum(V)),
                # T = tanh((alpha*s+beta)/2) = 2*(sigmoid(alpha*s+beta)-0.5)
                hsum_f = consts.tile([P, EC], F32)
                hsum_b = consts.tile([P, EC], BF)
                csum_sb = consts.tile([1, H * E], BF)   # 0.5 * colsum(V)
                ones1_sb = consts.tile([1, P], BF)
                nc.vector.memset(ones1_sb[:], 1.0)

            # ---------------- phase A: gather + LN + transpose ----------------
            with (
                tc.tile_pool(name="e_pool", bufs=1) as e_pool,
                tc.tile_pool(name="h_pool", bufs=3) as h_pool,
                tc.tile_pool(name="st_pool", bufs=8) as st_pool,
            ):
                # per-chunk LN so chunk 0's h (and the first vhat groups)
                # flow while later chunks still gather
                for c in range(NCH):
                    e_t = e_pool.tile([P, 1, E], F32, tag=f"e{c}")
                    nc.gpsimd.dma_gather(
                        e_t[:], emb_d.ap(), idx_sb[:, 8 * c : 8 * (c + 1)],
                        P, P, E,
                    )
                    stt = st_pool.tile([P, 6], F32, tag="bn")
                    nc.vector.bn_stats(stt[:], e_t[:, 0, :])
                    # per-chunk stats tile: a shared [P, NCH, 2] tile caused
                    # false subtile deps that serialized chunk 0's normalize
                    # behind all later chunks' aggregates
                    mv = st_pool.tile([P, 2], F32, tag=f"mv{c}")
                    nc.vector.bn_aggr(mv[:], stt[:])
                    var_ap = mv[:, 1:2]
                    nc.scalar.activation(
                        out=var_ap, in_=var_ap,
                        func=mybir.ActivationFunctionType.Sqrt,
                        bias=eps_sb[:, 0:1], scale=1.0,
                    )
                    nc.vector.reciprocal(var_ap, var_ap)
                    h_t = h_pool.tile([P, E], BF)
                    nc.vector.tensor_scalar(
                        out=h_t[:], in0=e_t[:, 0, :],
                        scalar1=mv[:, 0:1], scalar2=mv[:, 1:2],
                        op0=mybir.AluOpType.subtract, op1=mybir.AluOpType.mult,
                    )
                    # hT[p, c2, 128c+s] = h_t[s, 128*c2+p] via xbar transpose
                    nc.sync.dma_start_transpose(
                        hT_sb[:, :, c * P : (c + 1) * P], h_t[:]
                    )

            # ---------------- phases B+C: projections + attention ----------
            with (
                tc.tile_pool(name="sc_psum", bufs=3, space="PSUM") as sc_psum,
                tc.tile_pool(name="out_psum", bufs=2, space="PSUM") as out_psum,
                tc.tile_pool(name="sstat", bufs=10) as sstat,
                tc.tile_pool(name="w_pool", bufs=4) as w_pool,
                tc.tile_pool(name="wraw_pool", bufs=20) as wraw_pool,
                tc.tile_pool(name="wtb_pool", bufs=wtb_bufs) as wtb_pool,
                tc.tile_pool(name="wt_pool", bufs=16) as wt_pool,
                tc.tile_pool(name="ob_pool", bufs=6) as ob_pool,
            ):
                dum_bf = sstat.tile([P, S], BF, tag="dum", bufs=1)

                def vhat_group(c):
                    # V projection for t-group c, interleaved into phase C
                    # to keep TensorE dense.
                    for vp in range(4):
                        pv = sc_psum.tile([P, S], F32, tag="sc")
                        for c2 in range(EC):
                            lhsT = hT_sb[:, c2, c * P : (c + 1) * P]
                            for nn in range(2):
                                lo = vp * 1024 + nn * 512
                                nc.tensor.matmul(
                                    pv[:, nn * 512 : (nn + 1) * 512],
                                    lhsT,
                                    wvt_sb[:, c2, lo : lo + 512],
                                    start=(c2 == 0), stop=(c2 == EC - 1),
                                )
                        if use_fp8:
                            nc.scalar.mul(
                                vh_sb[:, c, vp * 1024 : (vp + 1) * 1024], pv[:], 0.5
                            )
                        else:
                            nc.scalar.copy(
                                vh_sb[:, c, vp * 1024 : (vp + 1) * 1024], pv[:]
                            )

                def out_stage(args, fast_store=False):
                    hp_, i_, wts = args
                    for sub in range(2):
                        h_idx = 2 * hp_ + sub
                        po = out_psum.tile([P, E], F32, tag="po")
                        if use_fp8:
                            # wts is one packed uint16 tile; fp8 view with
                            # byte b = sub: [p][cc][sub][s]
                            w4 = wts[:].bitcast(FP8).rearrange(
                                "p c (s two) -> p c two s", two=2
                            )
                            for j in range(4):
                                lhsT = w4[:, 2 * j : 2 * j + 2, sub, :]
                                rhs = vh_sb[
                                    :, 2 * j : 2 * j + 2,
                                    h_idx * E : (h_idx + 1) * E,
                                ]
                                nc.tensor.matmul(
                                    po[:], lhsT, rhs,
                                    start=(j == 0), stop=False,
                                    perf_mode=mybir.MatmulPerfMode.DoubleRow,
                                )
                            # += ones (x) 0.5*colsum(V): the centering term
                            nc.tensor.matmul(
                                po[:], ones1_sb[0:1, :],
                                csum_sb[0:1, h_idx * E : (h_idx + 1) * E],
                                start=False, stop=True,
                                skip_group_check=True,
                            )
                        else:
                            wt_t = wts[sub]
                            for cc in range(NCH):
                                nc.tensor.matmul(
                                    po[:],
                                    wt_t[:, cc, :],
                                    vh_sb[:, cc, h_idx * E : (h_idx + 1) * E],
                                    start=(cc == 0), stop=(cc == NCH - 1),
                                )
                        ob = ob_pool.tile([P, E], BF, tag="ob")
                        # alternate drain engines so two po tiles recycle
                        # in parallel
                        if sub == 0:
                            nc.scalar.copy(ob[:], po[:])
                        else:
                            nc.vector.tensor_copy(ob[:], po[:])
                        # SWDGE store keeps the sync queue free for the
                        # latency-critical wt transposes; the tail drain uses
                        # HWDGE for its shorter completion latency
                        if fast_store:
                            nc.sync.dma_start(
                                out_d[h_idx, i_ * P : (i_ + 1) * P, :], ob[:]
                            )
                        else:
                            nc.gpsimd.dma_start(
                                out_d[h_idx, i_ * P : (i_ + 1) * P, :], ob[:]
                            )

                # early V-hat groups: each needs only its own hT chunk, so
                # these overlap the tail of phase A and warm the PE
                for c in range(3):
                    vhat_group(c)

                # projections q/k
                for name, w_sb, t_sb in (
                    ("q", wqt_sb, qT_sb),
                    ("k", wkt_sb, kT_sb),
                ):
                    for sl in range(4):  # 128-wide (h,a) slices = head pairs
                        pq = sc_psum.tile([P, S], F32, tag="sc")
                        for c2 in range(EC):
                            lhsT = w_sb[:, c2, sl * P : (sl + 1) * P]
                            for nn in range(2):
                                nc.tensor.matmul(
                                    pq[:, nn * 512 : (nn + 1) * 512],
                                    lhsT,
                                    hT_sb[:, c2, nn * 512 : (nn + 1) * 512],
                                    start=(c2 == 0), stop=(c2 == EC - 1),
                                )
                        if use_beta:
                            bb = qb_sb if name == "q" else kb_sb
                            nc.vector.tensor_scalar(
                                out=t_sb[:, sl, :], in0=pq[:],
                                scalar1=bb[:, sl : sl + 1], scalar2=None,
                                op0=mybir.AluOpType.add, op1=mybir.AluOpType.bypass,
                            )
                        else:
                            nc.scalar.copy(t_sb[:, sl, :], pq[:])

                if use_fp8:
                    # 0.5*colsum(V)[v] = 0.5 * sum_e (sum_t h[t,e]) * WvT[e,v]
                    nc.vector.tensor_reduce(
                        hsum_f[:], hT_sb[:], axis=mybir.AxisListType.X,
                        op=mybir.AluOpType.add,
                    )
                    nc.vector.tensor_copy(hsum_b[:], hsum_f[:])
                    for j in range(8):
                        cs_ps = out_psum.tile([1, 512], F32, tag="po")
                        for c2 in range(EC):
                            nc.tensor.matmul(
                                cs_ps[:],
                                hsum_b[:, c2 : c2 + 1],
                                wvt_sb[:, c2, j * 512 : (j + 1) * 512],
                                start=(c2 == 0), stop=(c2 == EC - 1),
                            )
                        nc.scalar.mul(
                            csum_sb[0:1, j * 512 : (j + 1) * 512], cs_ps[:], 0.5
                        )

                pending = []
                unit_no = 0
                # pairs: groups of 2 query-chunk units sharing one stats batch
                for q in range(16):
                    hp = q // 4
                    i0 = 2 * (q % 4)
                    st = sstat.tile([P, 2, 8], F32, tag="st")
                    # st[:, u, :]: [mxA, mxB, mnA, mnB, betA, betB, alpA, alpB]
                    wraws = {}
                    for u in range(2):
                        i = i0 + u
                        # spread the remaining V-hat groups over units 2..7:
                        # early units are PE-rich (scores + no out-stages yet),
                        # late units starve, and all groups must finish before
                        # the first out_stage pop (unit ~9)
                        vg = {2: 3, 4: 4, 5: 5, 6: 6, 7: 7}.get(unit_no)
                        if vg is not None:
                            vhat_group(vg)
                        unit_no += 1
                        ps = []
                        for sub in range(2):
                            p0 = sub * 64
                            psc = sc_psum.tile([P, S], F32, tag="sc")
                            for nn in range(2):
                                nc.tensor.matmul(
                                    psc[:, nn * 512 : (nn + 1) * 512],
                                    qT_sb[p0 : p0 + 64, hp, i * P : (i + 1) * P],
                                    kT_sb[p0 : p0 + 64, hp, nn * 512 : (nn + 1) * 512],
                                    start=True, stop=True,
                                )
                            ps.append(psc)
                        for sub in range(2):
                            wraw = wraw_pool.tile([P, S], BF, tag="wr")
                            # fused PSUM->SBUF copy + row-max accum
                            nc.vector.tensor_scalar(
                                out=wraw[:], in0=ps[sub][:],
                                scalar1=-3.0e38, scalar2=None,
                                op0=mybir.AluOpType.max, op1=mybir.AluOpType.max,
                                accum_out=st[:, u, sub : sub + 1],
                            )
                            wraws[(u, sub)] = wraw
                        for sub in range(2):
                            # two-stage row-min: bf16 TT-min of halves (2x),
                            # then min-accum over the 512-wide result
                            wr = wraws[(u, sub)]
                            nc.vector.tensor_tensor(
                                out=dum_bf[:, 0:512], in0=wr[:, 0:512],
                                in1=wr[:, 512:1024], op=mybir.AluOpType.min,
                            )
                            nc.vector.tensor_scalar(
                                out=dum_bf[:, 512:1024], in0=dum_bf[:, 0:512],
                                scalar1=3.0e38, scalar2=None,
                                op0=mybir.AluOpType.min, op1=mybir.AluOpType.min,
                                accum_out=st[:, u, 2 + sub : 3 + sub],
                            )
                        keep = keep_pending if q < 14 else 2
                        while len(pending) > keep:
                            out_stage(pending.pop(0))
                    # batched alpha/beta for the 4 subs of this pair batch:
                    # sigmoid: alpha = 10/(mx-mn); beta = -mn*alpha - 5
                    # tanh/2 (fp8): alpha = 5/(mx-mn); beta = -mn*alpha - 2.5
                    amul = 5.0 if use_fp8 else 10.0
                    boff = -2.5 if use_fp8 else -5.0
                    mx = st[:, :, 0:2]
                    mn = st[:, :, 2:4]
                    bet = st[:, :, 4:6]
                    alp = st[:, :, 6:8]
                    nc.vector.tensor_tensor(
                        out=alp, in0=mx, in1=mn, op=mybir.AluOpType.subtract
                    )
                    nc.vector.reciprocal(alp, alp)
                    nc.vector.tensor_scalar_mul(alp, alp, amul)
                    nc.vector.tensor_tensor(
                        out=bet, in0=mn, in1=alp, op=mybir.AluOpType.mult
                    )
                    nc.vector.tensor_scalar(
                        out=bet, in0=bet, scalar1=-1.0, scalar2=boff,
                        op0=mybir.AluOpType.mult, op1=mybir.AluOpType.add,
                    )
                    act_fn = (
                        mybir.ActivationFunctionType.Tanh if use_fp8
                        else mybir.ActivationFunctionType.Sigmoid
                    )
                    for u in range(2):
                        i = i0 + u
                        if use_fp8:
                            # both subs' tanh outputs packed as the two bytes
                            # of one uint16 tile; one transpose, no casts
                            pk = w_pool.tile([P, S], I16, tag="pk")
                            pk8 = pk[:].bitcast(FP8).rearrange(
                                "p (s two) -> p two s", two=2
                            )
                            for sub in range(2):
                                nc.scalar.activation(
                                    out=pk8[:, sub, :], in_=wraws[(u, sub)][:],
                                    func=act_fn,
                                    bias=st[:, u, 4 + sub : 5 + sub],
                                    scale=st[:, u, 6 + sub : 7 + sub],
                                )
                            wtp = wt_pool.tile([P, NCH, P], I16, tag="wt")
                            nc.sync.dma_start_transpose(wtp[:], pk[:])
                            pending.append((hp, i, wtp))
                        else:
                            wts = []
                            for sub in range(2):
                                w_t = w_pool.tile([P, S], BF, tag="w")
                                nc.scalar.activation(
                                    out=w_t[:], in_=wraws[(u, sub)][:],
                                    func=act_fn,
                                    bias=st[:, u, 4 + sub : 5 + sub],
                                    scale=st[:, u, 6 + sub : 7 + sub],
                                )
                                wtb = wtb_pool.tile([P, NCH, P], BF, tag="wtb")
                                nc.sync.dma_start_transpose(wtb[:], w_t[:])
                                wts.append(wtb)
                            pending.append((hp, i, wts))
                for pp_ in pending:
                    out_stage(pp_, fast_store=True)

    nc.compile()
    _BUILD_CACHE[key] = nc
    return nc


def _prep_inputs(x, emb_table, gamma, beta, Wq, Wk, Wv, use_beta):
    x = np.asarray(x)
    gamma = np.asarray(gamma, dtype=np.float32)
    beta = np.asarray(beta, dtype=np.float32)
    Wq = np.asarray(Wq, dtype=np.float32)
    Wk = np.asarray(Wk, dtype=np.float32)
    Wv = np.asarray(Wv, dtype=np.float32)
    emb = np.ascontiguousarray(np.asarray(emb_table, dtype=np.float32))

    # W'[h,a,e] = W[h,a,e] * gamma[e]; layouts [e, h*ad+a] with the
    # device e-permutation e_dev[p, c2] = e_logical[4p + c2] handled by
    # the rearrange on the DMA load (row-major (p, ec) blocks), so host
    # just provides [e, j] with e in logical order re-grouped as 4p+c2:
    # row index r = 4p + c2 must equal logical e  ->  identity.
    wqt = np.ascontiguousarray(
        (Wq * gamma[None, None, :]).reshape(H * A, E).T.astype(BF16)
    )
    wkt = np.ascontiguousarray(
        (Wk * gamma[None, None, :]).reshape(H * A, E).T.astype(BF16)
    )
    wvt = np.ascontiguousarray(
        (Wv * gamma[None, None, :]).reshape(H * E, E).T.astype(BF16)
    )

    consts = dict(emb=emb, wqt=wqt, wkt=wkt, wvt=wvt)
    if use_beta:
        qb = (Wq.reshape(H * A, E) @ beta).astype(np.float32)   # [512]
        kb = (Wk.reshape(H * A, E) @ beta).astype(np.float32)
        vb = (Wv.reshape(H * E, E) @ beta).astype(np.float32)   # [4096]
        consts["qb"] = np.ascontiguousarray(qb.reshape(4, P).T)
        consts["kb"] = np.ascontiguousarray(kb.reshape(4, P).T)
        consts["vb"] = vb.reshape(1, H * E)

    in_maps = []
    for b in range(B):
        xi = x[b].astype(np.int64)
        idx16 = np.ascontiguousarray(
            xi.reshape(S // 16, 16).T.astype(np.int16)
        )  # [16, 64]; token j of chunk c sits at [j%16, 8c + j//16]
        idx_full = np.ascontiguousarray(np.tile(idx16, (8, 1)))  # [128, 64]
        in_maps.append(dict(idx=idx_full, **consts))
    return in_maps


def kernel(x, emb_table, gamma, beta, Wq, Wk, Wv):
    global LAST_RESULTS
    beta_arr = np.asarray(beta, dtype=np.float32)
    use_beta = bool(np.any(beta_arr != 0.0))

    nc = build_nc(use_beta, USE_FP8_OUT)
    in_maps = _prep_inputs(x, emb_table, gamma, beta, Wq, Wk, Wv, use_beta)

    trace = os.environ.get("KERNEL_TRACE", "0") == "1"
    res = run_bass_kernel_spmd(
        nc, in_maps, core_ids=list(range(B)), trace=trace
    )
    LAST_RESULTS = res

    out = np.stack([np.asarray(res.results[b]["out"]) for b in range(B)], axis=0)
    return out.astype(np.float32)


if __name__ == "__main__":
    rng = np.random.default_rng(0)
    x = rng.integers(0, VOCAB, size=(B, S), dtype=np.int32)
    emb = rng.standard_normal((VOCAB, E), dtype=np.float32)
    gamma = np.ones(E, np.float32)
    beta = np.zeros(E, np.float32)
    Wq = rng.random((H, A, E), dtype=np.float32)
    Wk = rng.random((H, A, E), dtype=np.float32)
    Wv = rng.random((H, E, E), dtype=np.float32)
    out = kernel(x, emb, gamma, beta, Wq, Wk, Wv)
    print(out.shape, out.dtype)
